# revision 10
# baseline (speedup 1.0000x reference)
"""ForgetMult (h_t = f_t*h_{t-1} + (1-f_t)*z_t) on 8 TRN2 NeuronCores.

Full inputs f, z: [T=1024, B=32, H=1024] f32. Output h: [T, B, H] f32.

Sharding: batch dim across the 8 cores (4 batches/core), no communication.
Per core: independent linear recurrence along T for N = 4096 columns.

v5: all-DVE pipeline (measured: PE at low pstate + ACT copies + extra
sync cost MORE than they save; GPSIMD rejects TensorScalarPtr ops).
  - HOST uploads f, z transposed to [N, T] fp16, reads h back [N, T]
    fp16 (24 MiB/core ~= 72 us DMA floor at the ~349 GB/s/core measured)
  - per group: STT bneg = (f-1)*z (1.07 ns/elem) + per-block scans
    (2.2 ns/elem, len-1024 measured fastest)
  - group sizes ramp 1,1,2,4,...,4,2,1,1 to shrink pipeline head/tail
  - t=0 host fixup (f'_0=0, z'_0=(1-f_0)z_0) keeps every column
    self-resetting so scans may chain across blocks when profitable
  - groups 5/6/7 carry scan-configuration probes (fp32 data0 / chained
    2048 / all-fp32) to measure dtype/length effects on the scan rate

Precision: fp16 I/O quantization, fp32 scan state -> rel err ~5e-4.
"""

from contextlib import ExitStack

import numpy as np

T, B, H = 1024, 32, 1024
NCORES = 8
BPC = B // NCORES  # 4 batches per core
N = BPC * H  # 4096 recurrence columns per core
P = 128

JMAX = 4
GROUPS = [1, 1, 2, 4, 4, 4, 4, 4, 4, 2, 1, 1]  # blocks per group, sum 32
assert sum(GROUPS) == N // P


def build_forget_mult(tc, h_d, f_d, z_d, ctx):
    """Emit the per-core Tile program. f_d/z_d/h_d are DRAM APs [N, T] fp16."""
    from concourse import mybir

    nc = tc.nc
    f16 = mybir.dt.float16
    su = mybir.AluOpType.subtract
    mu = mybir.AluOpType.mult

    f_pool = ctx.enter_context(tc.tile_pool(name="fpanel", bufs=4))
    z_pool = ctx.enter_context(tc.tile_pool(name="zpanel", bufs=4))
    b_pool = ctx.enter_context(tc.tile_pool(name="bpanel", bufs=3))
    h_pool = ctx.enter_context(tc.tile_pool(name="hpanel", bufs=3))
    t_pool = ctx.enter_context(tc.tile_pool(name="tpanel", bufs=2))

    def group_dram(d, r0, gj):
        # rows [P*r0 : P*(r0+gj)] of [N, T] viewed as [p, j, t]
        return d[P * r0 : P * (r0 + gj), :].rearrange("(j p) t -> p j t", p=P)

    r0 = 0
    for g, gj in enumerate(GROUPS):
        fp = f_pool.tile([P, gj, T], f16, tag="fpanel", name=f"fp{g}")
        nc.sync.dma_start(fp[:], group_dram(f_d, r0, gj))
        zp = z_pool.tile([P, gj, T], f16, tag="zpanel", name=f"zp{g}")
        nc.sync.dma_start(zp[:], group_dram(z_d, r0, gj))
        hp = h_pool.tile([P, gj, T], f16, tag="hpanel", name=f"hp{g}")

        # bneg = (f-1)*z. For full-size groups, GPSIMD (otherwise idle)
        # computes the first half as f*z - z (two Pool tensor_tensor ops,
        # verified on HW) while DVE does the rest as one STT -- this takes
        # ~2.2 us/group off the DVE critical path. Scans then run chained
        # in pairs of blocks (len-2048 measured fastest at 2.147 ns/elem;
        # 1024 = 2.24, 4096 = 2.55). Column starts self-reset via the host
        # t=0 fixup, so chains are exact.
        bp = b_pool.tile([P, gj, T], f16, tag="bpanel", name=f"bp{g}")
        if gj >= 4:
            gh = gj // 2
            tq = t_pool.tile([P, gh, T], f16, tag="tpanel", name=f"tq{g}")
            nc.gpsimd.tensor_tensor(tq[:], fp[:, :gh], zp[:, :gh], op=mu)
            nc.gpsimd.tensor_tensor(bp[:, :gh], tq[:], zp[:, :gh], op=su)
            nc.vector.scalar_tensor_tensor(
                bp[:, gh:], fp[:, gh:], 1.0, zp[:, gh:], op0=su, op1=mu
            )
        else:
            nc.vector.scalar_tensor_tensor(
                bp[:], fp[:], 1.0, zp[:], op0=su, op1=mu
            )
        j0 = 0
        while j0 < gj:
            cw = 2 if gj - j0 >= 2 else 1
            # state = (f * state) - bneg == f*state + (1-f)*z ; fp32 state
            nc.vector.tensor_tensor_scan(
                hp[:, j0 : j0 + cw].rearrange("p j t -> p (j t)"),
                fp[:, j0 : j0 + cw].rearrange("p j t -> p (j t)"),
                bp[:, j0 : j0 + cw].rearrange("p j t -> p (j t)"),
                0.0,
                op0=mu,
                op1=su,
            )
            j0 += cw
        nc.sync.dma_start(group_dram(h_d, r0, gj), hp[:])
        r0 += gj


def build_program():
    import concourse.tile as tile
    from concourse import bacc, mybir

    nc = bacc.Bacc(
        "TRN2",
        target_bir_lowering=False,
        debug=False,
        enable_asserts=False,
        num_devices=NCORES,
    )
    f16 = mybir.dt.float16
    f_d = nc.dram_tensor("f", [N, T], f16, kind="ExternalInput").ap()
    z_d = nc.dram_tensor("z", [N, T], f16, kind="ExternalInput").ap()
    h_d = nc.dram_tensor("h", [N, T], f16, kind="ExternalOutput").ap()
    with tile.TileContext(nc) as tc:
        with ExitStack() as ctx:
            build_forget_mult(tc, h_d, f_d, z_d, ctx)
    nc.compile()
    return nc


_compiled = None


def _get_program():
    global _compiled
    if _compiled is None:
        _compiled = build_program()
    return _compiled


def kernel(f, z, _trace=False):
    from concourse.bass_utils import run_bass_kernel_spmd

    f = np.asarray(f, dtype=np.float32)
    z = np.asarray(z, dtype=np.float32)
    assert f.shape == (T, B, H) and z.shape == (T, B, H)

    nc = _get_program()
    in_maps = []
    for c in range(NCORES):
        # [T, BPC, H] -> [T, N] -> transpose -> [N, T], downcast to fp16
        fc = f[:, c * BPC : (c + 1) * BPC, :].reshape(T, N).T
        zc = z[:, c * BPC : (c + 1) * BPC, :].reshape(T, N).T
        fc16 = np.ascontiguousarray(fc, dtype=np.float16)
        zc16 = np.ascontiguousarray(zc, dtype=np.float16)
        # t=0 fixup: f'_0 = 0, z'_0 = (1-f_0)*z_0: bneg_0 = -(1-f_0)z_0 and
        # h_0 = 0*carry - bneg_0 is exact for any carried state, so scans
        # may chain across column boundaries.
        z0 = (1.0 - fc[:, 0]) * zc[:, 0]  # fp32 math
        fc16[:, 0] = np.float16(0.0)
        zc16[:, 0] = z0.astype(np.float16)
        in_maps.append({"f": fc16, "z": zc16})

    kres = run_bass_kernel_spmd(nc, in_maps, list(range(NCORES)), trace=_trace)
    out = np.empty((T, B, H), dtype=np.float32)
    for c in range(NCORES):
        hc = kres.results[c]["h"]  # [N, T] fp16
        out[:, c * BPC : (c + 1) * BPC, :] = (
            hc.astype(np.float32).reshape(BPC, H, T).transpose(2, 0, 1)
        )
    if _trace:
        return out, kres
    return out


# revision 14
# speedup vs baseline: 1.2794x; 1.2794x over previous
"""ForgetMult (h_t = f_t*h_{t-1} + (1-f_t)*z_t) on 8 TRN2 NeuronCores.

Full inputs f, z: [T=1024, B=32, H=1024] f32. Output h: [T, B, H] f32.

Sharding: batch dim across the 8 cores (4 batches/core), no communication.
Per core: independent linear recurrence along T for N = 4096 columns.

v5: all-DVE pipeline (measured: PE at low pstate + ACT copies + extra
sync cost MORE than they save; GPSIMD rejects TensorScalarPtr ops).
  - HOST uploads f, z transposed to [N, T] fp16, reads h back [N, T]
    fp16 (24 MiB/core ~= 72 us DMA floor at the ~349 GB/s/core measured)
  - per group: STT bneg = (f-1)*z (1.07 ns/elem) + per-block scans
    (2.2 ns/elem, len-1024 measured fastest)
  - group sizes ramp 1,1,2,4,...,4,2,1,1 to shrink pipeline head/tail
  - t=0 host fixup (f'_0=0, z'_0=(1-f_0)z_0) keeps every column
    self-resetting so scans may chain across blocks when profitable
  - groups 5/6/7 carry scan-configuration probes (fp32 data0 / chained
    2048 / all-fp32) to measure dtype/length effects on the scan rate

Precision: fp16 I/O quantization, fp32 scan state -> rel err ~5e-4.
"""

from contextlib import ExitStack

import numpy as np

T, B, H = 1024, 32, 1024
NCORES = 8
BPC = B // NCORES  # 4 batches per core
N = BPC * H  # 4096 recurrence columns per core
P = 128

JMAX = 4
GROUPS = [1, 1, 2, 4, 4, 4, 4, 4, 4, 2, 1, 1]  # blocks per group, sum 32
assert sum(GROUPS) == N // P


def build_forget_mult(tc, h_d, f_d, z_d, ctx):
    """Emit the per-core Tile program. f_d/z_d/h_d are DRAM APs [N, T] fp16."""
    from concourse import mybir

    nc = tc.nc
    f16 = mybir.dt.float16
    su = mybir.AluOpType.subtract
    mu = mybir.AluOpType.mult

    f_pool = ctx.enter_context(tc.tile_pool(name="fpanel", bufs=4))
    z_pool = ctx.enter_context(tc.tile_pool(name="zpanel", bufs=4))
    b_pool = ctx.enter_context(tc.tile_pool(name="bpanel", bufs=3))
    h_pool = ctx.enter_context(tc.tile_pool(name="hpanel", bufs=3))

    def group_dram(d, r0, gj):
        # rows [P*r0 : P*(r0+gj)] of [N, T] viewed as [p, j, t]
        return d[P * r0 : P * (r0 + gj), :].rearrange("(j p) t -> p j t", p=P)

    r0 = 0
    for g, gj in enumerate(GROUPS):
        # f-loads issue on the Sync HWDGE queue, z-loads and h-stores on
        # the (otherwise idle) Activation HWDGE queue: descriptor gen is
        # ~1 us per DMA and serializes per queue, so splitting halves the
        # issue latency on the pipeline head and removes mid-stream
        # DMA-supply bubbles.
        fp = f_pool.tile([P, gj, T], f16, tag="fpanel", name=f"fp{g}")
        nc.sync.dma_start(fp[:], group_dram(f_d, r0, gj))
        zp = z_pool.tile([P, gj, T], f16, tag="zpanel", name=f"zp{g}")
        nc.scalar.dma_start(zp[:], group_dram(z_d, r0, gj))
        hp = h_pool.tile([P, gj, T], f16, tag="hpanel", name=f"hp{g}")

        # bneg = (f-1)*z, one DVE STT per group. (Offloading this to
        # GPSIMD was measured NET-NEGATIVE: Pool TT runs 2-4 ns/elem and,
        # worse, DVE and GPSIMD share SBUF ports -- concurrent Pool work
        # nearly halves DVE throughput. PE identity-matmul offload also
        # measured slower: low pstate + per-matmul LDWEIGHTS + sync tax.)
        bp = b_pool.tile([P, gj, T], f16, tag="bpanel", name=f"bp{g}")
        nc.vector.scalar_tensor_tensor(bp[:], fp[:], 1.0, zp[:], op0=su, op1=mu)
        j0 = 0
        while j0 < gj:
            cw = 2 if gj - j0 >= 2 else 1
            # state = (f * state) - bneg == f*state + (1-f)*z ; fp32 state
            nc.vector.tensor_tensor_scan(
                hp[:, j0 : j0 + cw].rearrange("p j t -> p (j t)"),
                fp[:, j0 : j0 + cw].rearrange("p j t -> p (j t)"),
                bp[:, j0 : j0 + cw].rearrange("p j t -> p (j t)"),
                0.0,
                op0=mu,
                op1=su,
            )
            j0 += cw
        nc.scalar.dma_start(group_dram(h_d, r0, gj), hp[:])
        r0 += gj


def build_program():
    import concourse.tile as tile
    from concourse import bacc, mybir

    nc = bacc.Bacc(
        "TRN2",
        target_bir_lowering=False,
        debug=False,
        enable_asserts=False,
        num_devices=NCORES,
    )
    f16 = mybir.dt.float16
    f_d = nc.dram_tensor("f", [N, T], f16, kind="ExternalInput").ap()
    z_d = nc.dram_tensor("z", [N, T], f16, kind="ExternalInput").ap()
    h_d = nc.dram_tensor("h", [N, T], f16, kind="ExternalOutput").ap()
    with tile.TileContext(nc) as tc:
        with ExitStack() as ctx:
            build_forget_mult(tc, h_d, f_d, z_d, ctx)
    nc.compile()
    return nc


_compiled = None


def _get_program():
    global _compiled
    if _compiled is None:
        _compiled = build_program()
    return _compiled


def kernel(f, z, _trace=False):
    from concourse.bass_utils import run_bass_kernel_spmd

    f = np.asarray(f, dtype=np.float32)
    z = np.asarray(z, dtype=np.float32)
    assert f.shape == (T, B, H) and z.shape == (T, B, H)

    nc = _get_program()
    in_maps = []
    for c in range(NCORES):
        # [T, BPC, H] -> [T, N] -> transpose -> [N, T], downcast to fp16
        fc = f[:, c * BPC : (c + 1) * BPC, :].reshape(T, N).T
        zc = z[:, c * BPC : (c + 1) * BPC, :].reshape(T, N).T
        fc16 = np.ascontiguousarray(fc, dtype=np.float16)
        zc16 = np.ascontiguousarray(zc, dtype=np.float16)
        # t=0 fixup: f'_0 = 0, z'_0 = (1-f_0)*z_0: bneg_0 = -(1-f_0)z_0 and
        # h_0 = 0*carry - bneg_0 is exact for any carried state, so scans
        # may chain across column boundaries.
        z0 = (1.0 - fc[:, 0]) * zc[:, 0]  # fp32 math
        fc16[:, 0] = np.float16(0.0)
        zc16[:, 0] = z0.astype(np.float16)
        in_maps.append({"f": fc16, "z": zc16})

    kres = run_bass_kernel_spmd(nc, in_maps, list(range(NCORES)), trace=_trace)
    out = np.empty((T, B, H), dtype=np.float32)
    for c in range(NCORES):
        hc = kres.results[c]["h"]  # [N, T] fp16
        out[:, c * BPC : (c + 1) * BPC, :] = (
            hc.astype(np.float32).reshape(BPC, H, T).transpose(2, 0, 1)
        )
    if _trace:
        return out, kres
    return out
